# revision 36
# baseline (speedup 1.0000x reference)
"""Trainium2 Bass kernel for a dense CNN (conv trunk + SPP + 3 FC layers).

Sharding over 8 NeuronCores:
  - Conv trunk data-parallel over batch (8 images/core). Activations live in
    SBUF in a "strip" layout [C, H, B*W] (batch folded into width), bf16.
  - conv1 (7x7 s2 p3) is phase-decomposed on host: 2x2 input phases turn it
    into 16 taps of a 4x4 s1 conv over phase images. The host strip is
    column-phase-major ([3 pool phases][8 imgs][38 cols]) so the three
    pool-sibling output columns land in three PSUM banks; pool1's column
    max then runs as two contiguous DVE maxes in PSUM and one scalar
    bias+relu eviction (no full-res intermediate).
  - pooled1 is doubled along partitions (row h | row h+1) so conv2-L0's
    di taps stack to K=128.
  - FC stage tensor-parallel: AllGather(feats, fp8) -> fc1 (512 outs/core)
    -> PE transpose -> fc2 partials feature-major -> AllReduce -> bias+relu
    -> fc3 redundantly on every core.  SPP mean division is folded into fc1
    weights on host (feats additionally scaled by 1/8 for fp8 range).
"""

import sys

sys.path.insert(0, "/opt/trn_rl_repo")

import numpy as np
import ml_dtypes

import concourse.mybir as mybir
import concourse.tile as tile
from concourse import bacc
from concourse.bass_utils import run_bass_kernel_spmd

F32 = mybir.dt.float32
BF16 = mybir.dt.bfloat16
FP8 = mybir.dt.float8e4
NP_FP8 = ml_dtypes.float8_e4m3
NP_BF16 = ml_dtypes.bfloat16

RELU = mybir.ActivationFunctionType.Relu
COPY = mybir.ActivationFunctionType.Copy
MAX = mybir.AluOpType.max
ADD = mybir.AluOpType.add
MULT = mybir.AluOpType.mult
AXX = mybir.AxisListType.X

N_CORES = 8
B = 64
BL = B // N_CORES
W1S = 916        # conv1 strip row width: 3 phases * 304 + 4 pad
C1BAND = 9       # conv1 band rows (multiple of 3)
SPP_LEVELS = (6, 3, 2, 1)
FSCALE = 1.0


def _spp_bins():
    bins = []
    for L in SPP_LEVELS:
        bd = [((i * 10) // L, -((-(i + 1) * 10) // L)) for i in range(L)]
        for i0, i1 in bd:
            for j0, j1 in bd:
                bins.append((i0, i1, j0, j1))
    return bins


# ----------------------------------------------------------------------------
# device program
# ----------------------------------------------------------------------------

def build_program():
    nc = bacc.Bacc(None, target_bir_lowering=False)

    def din(name, shape, dt):
        return nc.dram_tensor(name, list(shape), dt, kind="ExternalInput")

    c1rep = din("c1rep", [96, 111, W1S], BF16)
    w1g = din("w1g", [2, 96, 64], BF16)
    b1 = din("b1", [64, 1], F32)
    w2_0 = din("w2_0", [128, 2, 128], BF16)
    w2s = [None] + [din(f"w2_{i}", [128, 2, 2, 128], BF16) for i in range(1, 4)]
    b2s = [din(f"b2_{i}", [128, 1], F32) for i in range(4)]
    w3s = [din(f"w3_{i}", [128 if i == 0 else 256, 2, 2, 256], BF16)
           for i in range(6)]
    b3s = [din(f"b3_{i}", [128, 2], F32) for i in range(6)]
    w1c = din("w1c", [128, 100, 512], BF16)
    b1c = din("b1c", [1, 512], BF16)
    w2c = din("w2c", [512, 4096], BF16)
    b2f2 = din("b2f2", [128, 32], BF16)
    w3h = [din(f"w3h_{i}", [128, 32, 500], BF16) for i in range(2)]
    b3f = din("b3f", [1, 1000], BF16)

    out = nc.dram_tensor("out", [64, 1000], F32, kind="ExternalOutput")

    ag_src = nc.dram_tensor("ag_src", [2, 128, BL, 50], BF16)
    ag_dst = nc.dram_tensor("ag_dst", [2, N_CORES, 128, BL, 50], BF16,
                            addr_space="Shared")
    ar_srcs = [nc.dram_tensor(f"ar_src{h}", [128, 16, B], BF16) for h in range(2)]
    ar_dsts = [nc.dram_tensor(f"ar_dst{h}", [128, 16, B], BF16,
                              addr_space="Shared") for h in range(2)]
    warm_src = nc.dram_tensor("warm_src", [1, 16], F32)
    warm_dst = nc.dram_tensor("warm_dst", [N_CORES, 16], F32, addr_space="Shared")
    warm2_src = nc.dram_tensor("warm2_src", [1, 16], F32)
    warm2_dst = nc.dram_tensor("warm2_dst", [1, 16], F32, addr_space="Shared")

    tc_cm = tile.TileContext(nc)
    tc = tc_cm.__enter__()

    const_cm = tc.tile_pool(name="const", bufs=1); const = const_cm.__enter__()
    w1_cm = tc.tile_pool(name="w1pool", bufs=8); w1pool = w1_cm.__enter__()
    psum_holder = {}

    def ps(name):
        return psum_holder["pool"].tile([128, 512], F32, name=name, tag="ps")

    # --- small resident constants -------------------------------------------
    w1sb = const.tile([96, 2, 64], BF16, name="w1sb")
    nc.sync.dma_start(w1sb[:], w1g[:].transpose((1, 0, 2)))
    b1sb = const.tile([64, 1], F32, name="b1sb")
    nc.sync.dma_start(b1sb[:], b1[:])
    # ========================================================================
    # conv1 + pool1
    # ========================================================================
    mid_cm = tc.tile_pool(name="midpool", bufs=1); midpool = mid_cm.__enter__()
    p1_cm = tc.tile_pool(name="p1pool", bufs=1); p1pool = p1_cm.__enter__()
    # pooled1 doubled along partitions: rows 0-63 = ch at row h,
    # rows 64-127 = ch at row h+1 (stacks conv2-L0's di taps to K=128)
    pooled1 = p1pool.tile([128, 38, 304], BF16, name="pooled1")
    nc.vector.memset(pooled1[:64, 37, :], 0.0)
    nc.vector.memset(pooled1[64:, 36, :], 0.0)
    nc.vector.memset(pooled1[64:, 37, :], 0.0)

    psum_cm = tc.tile_pool(name="psum", bufs=7, space="PSUM")
    psum_holder["pool"] = psum = psum_cm.__enter__()
    band_cm = tc.tile_pool(name="band", bufs=2); band_pool = band_cm.__enter__()
    r0 = 0
    evict_flip = 0
    while r0 < 111:
        nr = min(C1BAND, 111 - r0)
        rep = band_pool.tile([96, C1BAND, W1S], BF16, name="rep", tag="rep", bufs=3)
        for rr in range(0, nr, 3):
            rn = min(3, nr - rr)
            nc.sync.dma_start(rep[:, rr:rr + rn, :],
                              c1rep[:, r0 + rr:r0 + rr + rn, :])
        t1ph = band_pool.tile([64, C1BAND, 3, 304], BF16, name="t1ph", tag="t1ph")
        t1 = band_pool.tile([64, C1BAND, 304], BF16, name="t1", tag="t1")
        rep_flat = rep[:].rearrange("k h w -> k (h w)")
        for r in range(nr):
            for m in range(3):
                p = ps("p_c1")
                for g in range(2):
                    m2 = m + 2 * g
                    base = r * W1S + (m2 % 3) * 304 + m2 // 3
                    nc.tensor.matmul(p[:64, :304], w1sb[:, g, :],
                                     rep_flat[:, base:base + 304],
                                     start=(g == 0), stop=(g == 1))
                # bias+relu commute with the pool max: evict activated
                dstv = t1ph[:, r, m, :]
                if evict_flip % 3 != 2:
                    nc.scalar.activation(dstv, p[:64, :304], RELU, bias=b1sb[:])
                else:
                    nc.vector.tensor_scalar(dstv, p[:64, :304],
                                            b1sb[:], 0.0, ADD, MAX)
                evict_flip += 1
        # pool1 column max over the 3 phases (contiguous bf16)
        nc.vector.tensor_tensor(t1[:, :nr, :], t1ph[:, :nr, 0, :],
                                t1ph[:, :nr, 1, :], MAX)
        nc.vector.tensor_tensor(t1[:, :nr, :], t1[:, :nr, :],
                                t1ph[:, :nr, 2, :], MAX)
        pr0, prn = r0 // 3, nr // 3
        pv = pooled1[:64, pr0:pr0 + prn, :]
        nc.vector.tensor_tensor(pv, t1[:, 0:3 * prn:3, :], t1[:, 1:3 * prn:3, :],
                                MAX)
        nc.vector.tensor_tensor(pv, pv, t1[:, 2:3 * prn:3, :], MAX)
        # shifted copy into partitions 64-127 (next-row view for conv2 L0)
        s0 = pr0 if pr0 > 0 else 1
        nc.sync.dma_start(pooled1[64:, s0 - 1:pr0 + prn - 1, :],
                          pooled1[:64, s0:pr0 + prn, :])
        r0 += nr
        if r0 == C1BAND:
            # emit the late-phase constant loads after band 0 is in flight
            w2sb, b2sb = [], []
            for i in range(4):
                if i == 0:
                    t = const.tile([128, 2, 128], BF16, name="w2sb0")
                    nc.sync.dma_start(t[:], w2_0[:])
                else:
                    t = const.tile([128, 2, 2, 128], BF16, name=f"w2sb{i}")
                    nc.sync.dma_start(t[:], w2s[i][:])
                w2sb.append(t)
                tb = const.tile([128, 1], F32, name=f"b2sb{i}")
                nc.sync.dma_start(tb[:], b2s[i][:])
                b2sb.append(tb)
            b3sb = []
            for i in range(6):
                tb = const.tile([128, 2], F32, name=f"b3sb{i}")
                nc.sync.dma_start(tb[:], b3s[i][:])
                b3sb.append(tb)
            ones_bf = const.tile([1, 64], BF16, name="ones_bf")
            nc.vector.memset(ones_bf[:], 1.0)
            b1csb = const.tile([1, 512], BF16, name="b1csb")
            nc.sync.dma_start(b1csb[:], b1c[:])
            b2f2sb = const.tile([128, 32], BF16, name="b2f2sb")
            nc.sync.dma_start(b2f2sb[:], b2f2[:])
            b3fsb = const.tile([1, 1000], BF16, name="b3fsb")
            nc.sync.dma_start(b3fsb[:], b3f[:])
            # warm up the collectives firmware with a tiny AllGather
            warm_sb = const.tile([1, 16], F32, name="warm_sb")
            nc.vector.memset(warm_sb[:], 0.0)
            nc.sync.dma_start(warm_src[:], warm_sb[:])
            nc.gpsimd.collective_compute(
                "AllGather", mybir.AluOpType.bypass,
                replica_groups=[list(range(N_CORES))],
                ins=[warm_src[:].opt()], outs=[warm_dst[:].opt()])
            nc.sync.dma_start(warm2_src[:], warm_sb[:])
            nc.gpsimd.collective_compute(
                "AllReduce", mybir.AluOpType.add,
                replica_groups=[list(range(N_CORES))],
                ins=[warm2_src[:].opt()], outs=[warm2_dst[:].opt()])
    band_cm.__exit__(None, None, None)


    # ========================================================================
    # conv2 block (4 layers), strip width 38/img (col 37 garbage)
    # ========================================================================
    a2_cm = tc.tile_pool(name="a2pool", bufs=2); a2pool = a2_cm.__enter__()
    cur = pooled1
    for li in range(4):
        hout = 36 - li
        w = w2sb[li]
        dst = a2pool.tile([128, 37, 304], BF16, name=f"a2_{li}", tag="a2")
        nc.vector.memset(dst[:, hout, :], 0.0)
        cur_flat = cur[:].rearrange("c h w -> c (h w)")
        for r in range(hout):
            p = ps("p_c2")
            if li == 0:
                # pooled1 partitions already stack (di, cin) -> K=128
                for dj in range(2):
                    nc.tensor.matmul(p[:, :304], w[:, dj, :],
                                     cur_flat[:, r * 304 + dj:r * 304 + dj + 304],
                                     start=(dj == 0), stop=(dj == 1))
            else:
                k = 0
                for di in range(2):
                    for dj in range(2):
                        base = (r + di) * 304 + dj
                        nc.tensor.matmul(p[:, :304], w[:, di, dj, :],
                                         cur_flat[:, base:base + 304],
                                         start=(k == 0), stop=(k == 3))
                        k += 1
            dstv = dst[:, r, :]
            if evict_flip % 2 == 0:
                nc.scalar.activation(dstv, p[:, :304], RELU, bias=b2sb[li][:])
            else:
                nc.vector.tensor_scalar(dstv, p[:, :304],
                                        b2sb[li][:], 0.0, ADD, MAX)
            evict_flip += 1
        cur = dst

    # pool2: 2x2 s2 (33 valid rows, 33 cols/img) -> [128, 16, 8*16]
    pooled2 = midpool.tile([128, 17, 128], BF16, name="pooled2")
    nc.vector.memset(pooled2[:, 16, :], 0.0)
    t2 = a2pool.tile([128, 33, 128], BF16, name="t2", tag="t2", bufs=1)
    va = cur[:, :34, :].rearrange("c h (b w) -> c h b w", w=38)
    t2v = t2[:].rearrange("c h (b w) -> c h b w", w=16)
    for rr in range(0, 33, 9):
        rn = min(9, 33 - rr)
        nc.vector.tensor_tensor(t2v[:, rr:rr + rn], va[:, rr:rr + rn, :, 0:32:2],
                                va[:, rr:rr + rn, :, 1:33:2], MAX)
    for rr in range(0, 16, 4):
        nc.vector.tensor_tensor(pooled2[:, rr:rr + 4, :],
                                t2[:, 2 * rr:2 * rr + 8:2, :],
                                t2[:, 2 * rr + 1:2 * rr + 9:2, :], MAX)
    # skew sync: realign cores behind conv3 so the feats gather
    # doesn't absorb accumulated drift
    nc.sync.dma_start(warm_src[:], warm_sb[:])
    nc.gpsimd.collective_compute(
        "AllGather", mybir.AluOpType.bypass,
        replica_groups=[list(range(N_CORES))],
        ins=[warm_src[:].opt()], outs=[warm_dst[:].opt()])
    a2_cm.__exit__(None, None, None)
    p1_cm.__exit__(None, None, None)

    # fc1 weight tiles (all 13 prefetched; stream during conv3)
    w1tiles = []
    for g in range(13):
        nkt = 8 if g < 12 else 4
        wt = w1pool.tile([128, 8, 512], BF16, name="w1t", tag="w1t")
        nc.sync.dma_start(wt[:, :nkt, :], w1c[:, 8 * g:8 * g + nkt, :])
        w1tiles.append(wt)
    # fc2 weights prefetch (used after the AllGather)
    w2p_cm = tc.tile_pool(name="w2pool", bufs=1, side="right")
    w2pool = w2p_cm.__enter__()
    w2sb2 = w2pool.tile([128, 4, 4096], BF16, name="w2sb2")
    nc.sync.dma_start(w2sb2[:], w2c[:].rearrange("(kt p) m -> p kt m", p=128))

    # ========================================================================
    # conv3 block (6 layers), strip width 16/img
    # ========================================================================
    fe_cm = tc.tile_pool(name="fepool", bufs=1); fepool = fe_cm.__enter__()
    a3_cm = tc.tile_pool(name="a3pool", bufs=2); a3pool = a3_cm.__enter__()
    w3_cm = tc.tile_pool(name="w3pool", bufs=2); w3pool = w3_cm.__enter__()

    feats = fepool.tile([128, 2, BL, 50], BF16, name="feats")
    rs = fepool.tile([128, 2, 15, 128], F32, name="rs")

    def emit_spp(ct, h5):
        with nc.allow_low_precision(reason="SPP small-window sums"):
            row = lambda h: h5[:, ct, h, :]
            sl = lambda i: rs[:, ct, i, :]
            TT = nc.vector.tensor_tensor
            # true adaptive-pool row bins over H=10 (bins overlap):
            # L6: [0,2) [1,4) [3,5) [5,7) [6,9) [8,10) -> slots 0..5
            # L3: [0,4) [3,7) [6,10) -> slots 6..8
            # L2: [0,5) [5,10) -> slots 9,10 ; L1: [0,10) -> slot 11
            # temps: s23=r2+r3 (12), s67=r6+r7 (13), t79=r7+r8+r9 (14)
            TT(sl(0), row(0), row(1), ADD)          # b0=[0,2)
            TT(sl(12), row(2), row(3), ADD)         # s23
            TT(sl(1), row(1), sl(12), ADD)          # b1=[1,4)
            TT(sl(2), row(3), row(4), ADD)          # b2=[3,5)
            TT(sl(3), row(5), row(6), ADD)          # b3=[5,7)
            TT(sl(13), row(6), row(7), ADD)         # s67
            TT(sl(4), sl(13), row(8), ADD)          # b4=[6,9)
            TT(sl(5), row(8), row(9), ADD)          # b5=[8,10)
            TT(sl(6), sl(0), sl(12), ADD)           # c0=[0,4)
            TT(sl(7), sl(2), sl(3), ADD)            # c1=[3,7)
            TT(sl(8), sl(4), row(9), ADD)           # c2=[6,10)
            TT(sl(9), sl(6), row(4), ADD)           # d0=[0,5)
            TT(sl(14), row(7), sl(5), ADD)          # t79=[7,10)
            TT(sl(10), sl(3), sl(14), ADD)          # d1=[5,10)
            TT(sl(11), sl(9), sl(10), ADD)          # e=[0,10)

            def rbv(L, i):
                slot = {6: i, 3: 6 + i, 2: 9 + i, 1: 11}[L]
                return rs[:, ct, slot, :].rearrange("c (b w) -> c b w", w=16)

            kbase = 0
            for L in SPP_LEVELS:
                for i in range(L):
                    src = rbv(L, i)
                    for j in range(L):
                        j0, j1 = (j * 10) // L, -((-(j + 1) * 10) // L)
                        nc.vector.tensor_reduce(
                            feats[:, ct, :, kbase + i * L + j],
                            src[:, :, j0:j1], AXX, ADD)
                kbase += L * L
            nc.sync.dma_start(ag_src[ct], feats[:, ct, :, :])
        nc.gpsimd.collective_compute(
            "AllGather", mybir.AluOpType.bypass,
            replica_groups=[list(range(N_CORES))],
            ins=[ag_src[ct].opt()], outs=[ag_dst[ct].opt()])

    ev_state = [0]
    a3prev = None
    for li in range(6):
        nkt = 1 if li == 0 else 2
        hin = 16 - li
        hout = hin - 1
        wsb = w3pool.tile([128, nkt, 2, 2, 256], BF16, name=f"w3sb{li}", tag="w3s")
        for kt in range(nkt):
            nc.sync.dma_start(wsb[:, kt, :, :, :],
                              w3s[li][kt * 128:(kt + 1) * 128, :, :, :])
        dst = a3pool.tile([128, 2, 16, 128], BF16, name=f"a3_{li}", tag="a3")
        nc.vector.memset(dst[:, :, hout, :], 0.0)
        if li == 0:
            src_flat = [pooled2[:].rearrange("c h w -> c (h w)")]
        else:
            src_flat = [a3prev[:, kt, :, :].rearrange("c h w -> c (h w)")
                        for kt in range(2)]
        def do_tile(mt, r, nr):
            p = ps("p_c3")
            k = 0
            nmm = 4 * nkt
            for kt in range(nkt):
                for di in range(2):
                    for dj in range(2):
                        base = (r + di) * 128 + dj
                        nc.tensor.matmul(
                            p[:, :nr * 128],
                            wsb[:, kt, di, dj, mt * 128:(mt + 1) * 128],
                            src_flat[kt][:, base:base + nr * 128],
                            start=(k == 0), stop=(k == nmm - 1))
                        k += 1
            dv = dst[:, mt, r:r + nr, :].rearrange("c h w -> c (h w)")
            nonlocal_flip = ev_state[0]
            if nonlocal_flip % 2 == 0:
                nc.scalar.activation(dv, p[:, :nr * 128], RELU,
                                     bias=b3sb[li][:, mt:mt + 1])
            else:
                nc.vector.tensor_scalar(dv, p[:, :nr * 128],
                                        b3sb[li][:, mt:mt + 1], 0.0, ADD, MAX)
            ev_state[0] += 1

        rcs = []
        r = 0
        while r < hout:
            rcs.append((r, min(3, hout - r)))
            r += rcs[-1][1]
        if li == 5:
            for (r, nr) in rcs:
                do_tile(0, r, nr)
            emit_spp(0, dst)
            for (r, nr) in rcs:
                do_tile(1, r, nr)
            emit_spp(1, dst)
        else:
            for (r, nr) in rcs:
                do_tile(0, r, nr)
                do_tile(1, r, nr)
        a3prev = dst

    w3_cm.__exit__(None, None, None)
    a3_cm.__exit__(None, None, None)

    fe_cm.__exit__(None, None, None)
    mid_cm.__exit__(None, None, None)
    fc_cm = tc.tile_pool(name="fcpool", bufs=1); fcpool = fc_cm.__enter__()
    w3p_cm = tc.tile_pool(name="w3fpool", bufs=2); w3fpool = w3p_cm.__enter__()
    w3halves = []
    for hi in range(2):
        wt3 = w3fpool.tile([128, 32, 500], BF16, name="w3half", tag="w3half")
        nc.sync.dma_start(wt3[:], w3h[hi][:])
        w3halves.append(wt3)
    featg2 = fcpool.tile([128, 2, N_CORES, BL, 50], BF16, name="featg2")
    featg2k = fcpool.tile([128, 2, 50, B], BF16, name="featg2k")
    for ct in range(2):
        nc.sync.dma_start(featg2[:, ct, :, :, :],
                          ag_dst[ct].transpose((1, 0, 2, 3)))
        # k-major copy so fc1's stationary loads are contiguous
        nc.vector.tensor_copy(
            featg2k[:, ct, :, :],
            featg2[:, ct, :, :, :].rearrange("c cr im k -> c k (cr im)"))

    # ========================================================================
    # fc1: [64, 512] = feats_full.T @ w1c (+bias), relu
    # ========================================================================
    pf1 = psum.tile([64, 512], F32, name="pf1", tag="pf1", bufs=1)
    kt = 0
    for ct in range(2):
        for k in range(50):
            nc.tensor.matmul(pf1[:], featg2k[:, ct, k, :],
                             w1tiles[kt // 8][:, kt % 8, :],
                             start=(kt == 0), stop=False)
            kt += 1
    nc.tensor.matmul(pf1[:], ones_bf[:], b1csb[:], start=False, stop=True)
    f1 = fcpool.tile([64, 512], BF16, name="f1")
    nc.scalar.activation(f1[:], pf1[:], RELU)

    # transpose f1 -> f1T [128, 4, 64] bf16 via DMA transpose
    f1T = fcpool.tile([128, 4, 64], BF16, name="f1T")
    for t in range(4):
        nc.sync.dma_start_transpose(f1T[:, t, :], f1[:, 128 * t:128 * (t + 1)])

    # ========================================================================
    # fc2 partials (feature-major) -> AllReduce -> bias+relu
    # ========================================================================
    f2x = fcpool.tile([128, 32, B], BF16, name="f2x")
    part2 = f2pre = f2T = f2x
    for mg in range(4):  # 8 mt per psum bank
        p = ps("p_f2")
        for mi in range(8):
            mt = mg * 8 + mi
            for ktt in range(4):
                nc.tensor.matmul(p[:, 64 * mi:64 * mi + B],
                                 w2sb2[:, ktt, 128 * mt:128 * (mt + 1)],
                                 f1T[:, ktt, :], start=(ktt == 0),
                                 stop=(ktt == 3))
        if mg % 2 == 0:
            nc.scalar.activation(
                part2[:, 8 * mg:8 * mg + 8, :].rearrange("p m b -> p (m b)"),
                p[:], COPY)
        else:
            nc.vector.tensor_copy(
                part2[:, 8 * mg:8 * mg + 8, :].rearrange("p m b -> p (m b)"),
                p[:])
        if mg % 2 == 1:
            h = mg // 2
            nc.sync.dma_start(ar_srcs[h][:], part2[:, 16 * h:16 * h + 16, :])
            nc.gpsimd.collective_compute(
                "AllReduce", mybir.AluOpType.add,
                replica_groups=[list(range(N_CORES))],
                ins=[ar_srcs[h][:].opt()], outs=[ar_dsts[h][:].opt()])
    # bias (per feature = per (partition, mt)) + relu, pipelined per k-half
    for h0 in (0, 16):
        nc.sync.dma_start(f2pre[:, h0:h0 + 16, :], ar_dsts[h0 // 16][:])
        badd = b2f2sb[:, h0:h0 + 16].unsqueeze(2).broadcast_to((128, 16, B))
        nc.vector.tensor_tensor(f2T[:, h0:h0 + 16, :],
                                f2pre[:, h0:h0 + 16, :], badd, ADD)
        nc.vector.tensor_scalar(
            f2T[:, h0:h0 + 16, :].rearrange("p m b -> p (m b)"),
            f2T[:, h0:h0 + 16, :].rearrange("p m b -> p (m b)"),
            0.0, None, MAX)

    # ========================================================================
    # fc3 (full, redundant per core)
    # ========================================================================
    osb = fcpool.tile([64, 1000], F32, name="osb")
    for hi, (n0, nn) in enumerate(((0, 500), (500, 500))):
        wt = w3halves[hi]
        p = ps("p_f3")
        for ktt in range(32):
            nc.tensor.matmul(p[:64, :nn], f2T[:, ktt, :], wt[:, ktt, :],
                             start=(ktt == 0), stop=False)
        nc.tensor.matmul(p[:64, :nn], ones_bf[:], b3fsb[:, n0:n0 + nn],
                         start=False, stop=True)
        nc.scalar.activation(osb[:, n0:n0 + nn], p[:64, :nn], COPY)
        nc.sync.dma_start(out[:, n0:n0 + nn], osb[:, n0:n0 + nn])

    w3p_cm.__exit__(None, None, None)
    w2p_cm.__exit__(None, None, None)
    fc_cm.__exit__(None, None, None)
    w1_cm.__exit__(None, None, None)
    psum_cm.__exit__(None, None, None)
    const_cm.__exit__(None, None, None)
    tc_cm.__exit__(None, None, None)

    nc.compile()
    return nc


# ----------------------------------------------------------------------------
# host-side input preparation
# ----------------------------------------------------------------------------

def _prep_conv1(x):
    """x [B,3,224,224] fp32 -> per-core phase-major tap strips
    [96, 111, W1S] bf16: row layout [3 pool-phase][8 img][38 cols]."""
    Bb = x.shape[0]
    xpad = np.zeros((Bb, 3, 230, 230), np.float32)
    xpad[:, :, 3:227, 3:227] = x
    xph = np.empty((Bb, 2, 2, 3, 115, 115), np.float32)
    for p in range(2):
        for q in range(2):
            xph[:, p, q] = xpad[:, :, p:p + 229:2, q:q + 229:2]
    xph = xph.astype(NP_BF16)
    reps = []
    for c in range(N_CORES):
        ph = xph[c * BL:(c + 1) * BL]  # [8, 2, 2, 3, 115, 115]
        rep = np.zeros((96, 111, W1S), NP_BF16)
        k = 0
        for g01 in range(2):
            for a in range(4):
                # value at (tap row, out row i, phase m, img b, w) =
                #   xph[b,p,q,c][i+a, 3w + m + g01]
                blk = ph[:, :, :, :, a:a + 111, g01:]  # [8,2,2,3,111,115-g01]
                for m in range(3):
                    v = blk[..., m:m + 112:3]          # [8,2,2,3,111,<=38]
                    wv = v.shape[-1]
                    vt = np.transpose(v, (1, 2, 3, 4, 0, 5))  # [2,2,3,111,8,w]
                    dst = rep[k:k + 12, :, m * 304:(m + 1) * 304]
                    dst.reshape(12, 111, 8, 38)[:, :, :, :wv] = \
                        vt.reshape(12, 111, 8, wv)
                k += 12
        reps.append(rep)
    return reps


def _prep_w1(w1):
    """w1 [64,3,7,7] -> w1g [2 groups, 96, 64] bf16 (zero-padded taps)."""
    w1g = np.zeros((2, 96, 64), np.float32)
    for g in range(2):
        k = 0
        for g01 in range(2):
            for a in range(4):
                for p in range(2):
                    for q in range(2):
                        di = 2 * a + p
                        dj = 2 * (g01 + 2 * g) + q
                        for c in range(3):
                            if di <= 6 and dj <= 6:
                                w1g[g, k] = w1[:, c, di, dj]
                            k += 1
    return w1g.astype(NP_BF16)


def _spp_counts():
    cnt = np.empty(50, np.float32)
    for kk, (i0, i1, j0, j1) in enumerate(_spp_bins()):
        cnt[kk] = (i1 - i0) * (j1 - j0)
    return cnt


_CACHED = {}


def kernel(**inputs):
    if "nc" not in _CACHED:
        _CACHED["nc"] = build_program()
    nc = _CACHED["nc"]

    x = np.asarray(inputs["x"], np.float32)
    reps = _prep_conv1(x)
    w1gv = _prep_w1(np.asarray(inputs["w1"], np.float32))
    b1v = np.asarray(inputs["b1"], np.float32).reshape(64, 1)

    fc1_w = np.asarray(inputs["fc1_w"], np.float32)
    fc1_b = np.asarray(inputs["fc1_b"], np.float32)
    fc2_w = np.asarray(inputs["fc2_w"], np.float32)
    fc2_b = np.asarray(inputs["fc2_b"], np.float32)
    fc3_w = np.asarray(inputs["fc3_w"], np.float32)
    fc3_b = np.asarray(inputs["fc3_b"], np.float32)

    cnt = _spp_counts()
    w1s = fc1_w.reshape(4096, 256, 50) / cnt[None, None, :]
    # device feature d = (ct*50 + k)*128 + c128 -> channel ct*128+c128, bin k
    # w1c layout [128 c, 100 (ct,k), 512 out-slice]
    w1d = np.ascontiguousarray(
        w1s.reshape(4096, 2, 128, 50).transpose(2, 1, 3, 0))  # [128,2,50,4096]

    w2cT = fc2_w.T  # [4096(in rows), 4096(out cols)] -> slice rows per core
    w3T = np.ascontiguousarray(fc3_w.T)  # [4096, 1000]
    w3hv = [np.ascontiguousarray(
        w3T.reshape(32, 128, 1000).transpose(1, 0, 2)[:, :, 500 * i:500 * (i + 1)]
        ).astype(NP_BF16) for i in range(2)]
    b3fv = fc3_b.reshape(1, 1000).astype(NP_BF16)
    b2f2v = np.ascontiguousarray(fc2_b.reshape(32, 128).T).astype(NP_BF16)

    conv_w = {}
    for i in range(4):
        wv = np.asarray(inputs[f"w2_{i}"], np.float32)
        if i == 0:
            # [(di cin)=128, dj, cout]
            conv_w["w2_0"] = np.ascontiguousarray(
                wv.transpose(2, 1, 3, 0).reshape(128, 2, 128)).astype(NP_BF16)
        else:
            # [cin, di, dj, cout]
            conv_w[f"w2_{i}"] = np.ascontiguousarray(
                wv.transpose(1, 2, 3, 0)).astype(NP_BF16)
        conv_w[f"b2_{i}"] = np.asarray(inputs[f"b2_{i}"], np.float32).reshape(128, 1)
    for i in range(6):
        conv_w[f"w3_{i}"] = np.ascontiguousarray(
            np.asarray(inputs[f"w3_{i}"], np.float32).transpose(1, 2, 3, 0)
            ).astype(NP_BF16)
        conv_w[f"b3_{i}"] = np.ascontiguousarray(
            np.asarray(inputs[f"b3_{i}"], np.float32).reshape(2, 128).T)

    in_maps = []
    for c in range(N_CORES):
        sl = slice(512 * c, 512 * (c + 1))
        m = {
            "c1rep": reps[c],
            "w1g": w1gv,
            "b1": b1v,
            "w1c": np.ascontiguousarray(w1d[:, :, :, sl]).reshape(
                128, 100, 512).astype(NP_BF16),
            "b1c": fc1_b[sl].reshape(1, 512).astype(NP_BF16),
            "w2c": np.ascontiguousarray(w2cT[sl]).astype(NP_BF16),
            "b2f2": b2f2v,
            "w3h_0": w3hv[0],
            "w3h_1": w3hv[1],
            "b3f": b3fv,
        }
        m.update(conv_w)
        in_maps.append(m)

    res = run_bass_kernel_spmd(
        nc, in_maps, core_ids=list(range(N_CORES)),
        trace=bool(_CACHED.get("trace")), tmpdir=_CACHED.get("tmpdir"))
    _CACHED["last_result"] = res
    return np.asarray(res.results[0]["out"], np.float32)
